# revision 19
# baseline (speedup 1.0000x reference)
"""MiaoBlock (equivariant GNN message passing) on 8 TRN2 NeuronCores.

Strategy (edge-parallel, two SPMD launches, host does only index prep +
shard/unshard):
  Launch 1 (node update): edges bucketed by destination core/window
    (host argsort of idx_i). Each core gathers node[idx_j] rows via
    dma_gather, computes the 11-combo tensor-product messages in
    [channel, edge] layout (PE matmuls for Wrbf/Wmix), aggregates
    per-128-node windows with a one-hot S matmul (segment_sum on PE),
    applies gating nonlinearity + residual, returns its 1250-node slice.
  Launch 2 (edge update): positional edge shard; gather updated nodes,
    same message pipeline, per-edge gating + residual, returns edge rows.

Feature packing (832 = 64*(1+3+9)) used everywhere:
  row = [w0: c | w1: s*64+c | w2: (v*3+u)*64+c]
"""
import sys

sys.path.insert(0, "/opt/trn_rl_repo")

import numpy as np

import concourse.bass as bass
import concourse.mybir as mybir
import concourse.bacc as bacc
from concourse.tile import TileContext
from concourse import bass_utils

F32 = mybir.dt.float32
I16 = mybir.dt.int16
AF = mybir.ActivationFunctionType
ALU = mybir.AluOpType

C = 64
NB = 12
RMAX = 5.0
WIDTH = RMAX / NB
NORM = 16.0
N = 10000
E = 160000
NCORES = 8
NPC = N // NCORES          # 1250 nodes per core
NW = (NPC + 127) // 128    # 10 windows of 128 nodes
NPAD = NW * 128            # 1280
EB = 512                   # edges per compute tile
FEAT = 832
# combos (in_way, r_way, out_way) in reference order
COMBOS = [(i, r, o) for o in range(3) for i in range(3) for r in range(3)
          if (i + r - o) >= 0 and (i + r - o) % 2 == 0
          and (i + r - o) // 2 <= min(i, r)]
NK = len(COMBOS)  # 11
LAST_EXEC_NS = None
LAST_PROFILES = []
LAST_NCS = []
# 13 output planes: (o, t) -> flat feature offset t.. ; plane index per o
PLANE_OFF = {0: 0, 1: 1, 2: 4}  # plane id = PLANE_OFF[o] + t


def _pack_feat(t0, t1, t2):
    """[?,64],[?,64,3],[?,64,3,3] -> [?,832] rows in plane-major layout."""
    n = t0.shape[0]
    return np.concatenate(
        [t0,
         t1.transpose(0, 2, 1).reshape(n, 3 * C),
         t2.transpose(0, 2, 3, 1).reshape(n, 9 * C)], axis=1
    ).astype(np.float32)


def _unpack_feat(rows):
    n = rows.shape[0]
    t0 = rows[:, :C].copy()
    t1 = rows[:, C:4 * C].reshape(n, 3, C).transpose(0, 2, 1).copy()
    t2 = rows[:, 4 * C:].reshape(n, 3, 3, C).transpose(0, 3, 1, 2).copy()
    return t0, t1, t2


def _wrap16(idx, epad):
    """edge i's gather index at [i%16, i//16] int16, replicated to 128
    partitions (8 gpsimd cores x 16)."""
    w = idx.astype(np.int16).reshape(epad // 16, 16).T
    return np.ascontiguousarray(np.tile(w, (8, 1)))


def build_conv(epad, wsched, phase1):
    """Emit the Tile program for one conv phase. wsched: window id per
    128-edge subtile (phase1 only)."""
    nc = bacc.Bacc("TRN2", target_bir_lowering=False, debug=False)
    ntiles = epad // EB

    # ---- DRAM tensors ----
    nodes_all = nc.dram_tensor("nodes_all", [N, FEAT], F32, kind="ExternalInput")
    edgeT = nc.dram_tensor("edgeT", [FEAT, epad], F32, kind="ExternalInput")
    rijrows = nc.dram_tensor("rijrows", [epad, 3], F32, kind="ExternalInput")
    gidx = nc.dram_tensor("gidx", [128, epad // 16], I16, kind="ExternalInput")
    wrbf_d = nc.dram_tensor("wrbf", [NK, NB, C], F32, kind="ExternalInput")
    wmix_d = nc.dram_tensor("wmix", [NK, C, C], F32, kind="ExternalInput")
    wg_d = nc.dram_tensor("wg", [2, C, C], F32, kind="ExternalInput")
    ident_d = nc.dram_tensor("ident", [128, 128], F32, kind="ExternalInput")
    centers_d = nc.dram_tensor("centers", [128, NB], F32, kind="ExternalInput")
    if phase1:
        widx_d = nc.dram_tensor("widx", [128, epad // 128], F32, kind="ExternalInput")
        iota_d = nc.dram_tensor("iota", [128, 128], F32, kind="ExternalInput")
        noderes_d = nc.dram_tensor("noderes", [NPAD, FEAT], F32, kind="ExternalInput")
        bgb_d = nc.dram_tensor("bgb", [2, 128, C], F32, kind="ExternalInput")
        nodeout_d = nc.dram_tensor("nodeout", [NPAD, FEAT], F32, kind="ExternalOutput")
    else:
        bgc_d = nc.dram_tensor("bgc", [C, 2], F32, kind="ExternalInput")
        edgeout_d = nc.dram_tensor("edgeoutT", [FEAT, epad], F32, kind="ExternalOutput")

    with TileContext(nc) as tc:
        with (
            tc.tile_pool(name="const", bufs=1) as cpool,
            tc.tile_pool(name="stream", bufs=2) as spool,
            tc.tile_pool(name="work", bufs=1) as wpool,
            tc.tile_pool(name="ps", bufs=2, space="PSUM") as pspool,
            tc.tile_pool(name="psagg", bufs=1, space="PSUM") as papool,
        ):
            # ---- constants resident in SBUF ----
            ident = cpool.tile([128, 128], F32, tag="ident")
            nc.sync.dma_start(ident[:], ident_d[:])
            centers = cpool.tile([128, NB], F32, tag="centers")
            nc.sync.dma_start(centers[:], centers_d[:])
            wrbf = cpool.tile([NB, NK * C], F32, tag="wrbf")
            for k in range(NK):
                nc.sync.dma_start(wrbf[:, k * C:(k + 1) * C], wrbf_d[k, :, :])
            wmix = cpool.tile([C, NK * C], F32, tag="wmix")
            for k in range(NK):
                nc.sync.dma_start(wmix[:, k * C:(k + 1) * C], wmix_d[k, :, :])
            wg = cpool.tile([C, 2 * C], F32, tag="wg")
            for k in range(2):
                nc.sync.dma_start(wg[:, k * C:(k + 1) * C], wg_d[k, :, :])
            if phase1:
                iota = cpool.tile([128, 128], F32, tag="iota")
                nc.sync.dma_start(iota[:], iota_d[:])
                bgb = cpool.tile([128, 2 * C], F32, tag="bgb")
                for k in range(2):
                    nc.sync.dma_start(bgb[:, k * C:(k + 1) * C], bgb_d[k, :, :])
                aggrows = cpool.tile([128, NW, FEAT], F32, tag="aggrows")
                nc.vector.memset(aggrows[:], 0.0)
            else:
                bgc = cpool.tile([C, 2], F32, tag="bgc")
                nc.sync.dma_start(bgc[:], bgc_d[:])
            bpi2 = cpool.tile([128, 1], F32, tag="bpi2")
            nc.vector.memset(bpi2[:], float(np.pi / 2))

            # ---- edge loop ----
            for it in range(ntiles):
                e0 = it * EB
                # edge feature planes [64, EB] x13
                ept = [spool.tile([C, EB], F32, tag=f"ept{p}", name=f"ept{p}", bufs=1) for p in range(13)]
                for p in range(13):
                    nc.sync.dma_start(ept[p][:], edgeT[p * C:(p + 1) * C, e0:e0 + EB])
                # per-edge scalar prep per 128-subtile
                rbfT = wpool.tile([NB, EB], F32, tag="rbfT")
                rT = [wpool.tile([1, EB], F32, tag=f"rT{u}", name=f"rT{u}")
                      for u in range(3)]
                gath = [spool.tile([128, 1, FEAT], F32, tag=f"gath{st}", name=f"gath{st}", bufs=1) for st in range(4)]
                wcol = [None] * 4
                if phase1:
                    wcol4 = wpool.tile([128, 4], F32, tag="wcol4", bufs=3)
                    nc.sync.dma_start(wcol4[:], widx_d[:, it * 4:(it + 1) * 4])
                    wcol = [wcol4[:, st:st + 1] for st in range(4)]
                for st in range(4):
                    es = e0 + st * 128
                    rij = wpool.tile([128, 3], F32, tag=f"rij{st}", bufs=2)
                    nc.sync.dma_start(rij[:], rijrows[es:es + 128, :])
                    gi = wpool.tile([128, 8], I16, tag=f"gi{st}", bufs=2)
                    nc.sync.dma_start(gi[:], gidx[:, es // 16:es // 16 + 8])
                    nc.gpsimd.dma_gather(gath[st][:], nodes_all[:], gi[:],
                                         128, 128, FEAT)
                    # radial basis + rhat
                    sq = wpool.tile([128, 3], F32, tag=f"sq{st}")
                    nc.scalar.square(sq[:], rij[:])
                    d2 = wpool.tile([128, 1], F32, tag=f"d2{st}")
                    nc.vector.tensor_reduce(d2[:], sq[:], mybir.AxisListType.X, ALU.add)
                    d = wpool.tile([128, 1], F32, tag=f"d{st}")
                    nc.scalar.sqrt(d[:], d2[:])
                    deps = wpool.tile([128, 1], F32, tag=f"deps{st}")
                    nc.vector.tensor_scalar_add(deps[:], d[:], 1e-9)
                    rcp = wpool.tile([128, 1], F32, tag=f"rcp{st}")
                    nc.vector.reciprocal(rcp[:], deps[:])
                    rhat = wpool.tile([128, 3], F32, tag=f"rhat{st}")
                    nc.vector.tensor_scalar_mul(rhat[:], rij[:], rcp[:])
                    # rbf = exp(-((centers-d)/w)^2) * fc(d)
                    rb = wpool.tile([128, NB], F32, tag=f"rb{st}")
                    nc.vector.tensor_scalar(rb[:], centers[:, :], d[:], None, ALU.subtract)
                    nc.scalar.activation(rb[:], rb[:], AF.Square, scale=1.0 / WIDTH)
                    nc.scalar.activation(rb[:], rb[:], AF.Exp, scale=-1.0)
                    dc = wpool.tile([128, 1], F32, tag=f"dc{st}")
                    nc.vector.tensor_scalar_min(dc[:], d[:], RMAX)
                    fc = wpool.tile([128, 1], F32, tag=f"fc{st}")
                    nc.scalar.activation(fc[:], dc[:], AF.Sin,
                                         bias=bpi2[:], scale=float(-np.pi / RMAX))
                    nc.scalar.activation(fc[:], fc[:], AF.Copy, bias=0.5, scale=0.5)
                    nc.vector.tensor_scalar_mul(rb[:], rb[:], fc[:])
                    # transpose rbf and rhat into [·, EB] assembly tiles
                    tp = pspool.tile([NB, 128], F32, tag="tp")
                    nc.tensor.transpose(tp[:], rb[:], ident[:])
                    nc.scalar.copy(rbfT[:, st * 128:(st + 1) * 128], tp[:])
                    for u in range(3):
                        tp2 = pspool.tile([1, 128], F32, tag="tp", name="tp2")
                        nc.tensor.transpose(tp2[:], rhat[:, u:u + 1], ident[:])
                        nc.scalar.copy(rT[u][0:1, st * 128:(st + 1) * 128],
                                       tp2[:])
                # broadcast rhat rows -> [64, EB]
                R = [wpool.tile([C, EB], F32, tag=f"R{u}", name=f"R{u}") for u in range(3)]
                for u in range(3):
                    nc.gpsimd.partition_broadcast(R[u][:], rT[u][:])
                # f_k = (rbf @ Wrbf_k)^T : [64, EB]
                fk = [wpool.tile([C, EB], F32, tag=f"fk{k}", name=f"fk{k}") for k in range(NK)]
                for k in range(NK):
                    fp = pspool.tile([C, EB], F32, tag="mmps")
                    nc.tensor.matmul(fp[:], wrbf[:, k * C:(k + 1) * C], rbfT[:],
                                     start=True, stop=True)
                    nc.scalar.copy(fk[k][:], fp[:])
                # base planes = gathered nodes (transposed) + edge features
                base = [wpool.tile([C, EB], F32, tag=f"base{p}", name=f"base{p}") for p in range(13)]
                for st in range(4):
                    for p in range(13):
                        tp = pspool.tile([C, 128], F32, tag="tp")
                        nc.tensor.transpose(tp[:], gath[st][:, 0, p * C:(p + 1) * C],
                                            ident[:])
                        nc.vector.tensor_add(base[p][:, st * 128:(st + 1) * 128],
                                             tp[:], ept[p][:, st * 128:(st + 1) * 128])
                # shared contractions
                c11 = wpool.tile([C, EB], F32, tag="c11")
                tmp = wpool.tile([C, EB], F32, tag="ctmp")
                nc.vector.tensor_mul(c11[:], base[1][:], R[0][:])
                nc.vector.tensor_mul(tmp[:], base[2][:], R[1][:])
                nc.vector.tensor_add(c11[:], c11[:], tmp[:])
                nc.vector.tensor_mul(tmp[:], base[3][:], R[2][:])
                nc.vector.tensor_add(c11[:], c11[:], tmp[:])
                c21 = [wpool.tile([C, EB], F32, tag=f"c21{v}", name=f"c21{v}") for v in range(3)]
                for v in range(3):
                    nc.vector.tensor_mul(c21[v][:], base[4 + v * 3][:], R[0][:])
                    nc.vector.tensor_mul(tmp[:], base[4 + v * 3 + 1][:], R[1][:])
                    nc.vector.tensor_add(c21[v][:], c21[v][:], tmp[:])
                    nc.vector.tensor_mul(tmp[:], base[4 + v * 3 + 2][:], R[2][:])
                    nc.vector.tensor_add(c21[v][:], c21[v][:], tmp[:])

                # y planes per msg plane; matmul-accumulate into msg psum
                msg = [wpool.tile([C, EB], F32, tag=f"msg{p}", name=f"msg{p}") for p in range(13)]
                ya = wpool.tile([C, EB], F32, tag="ya")
                yb = wpool.tile([C, EB], F32, tag="yb")
                yc = wpool.tile([C, EB], F32, tag="yc")

                def mixmat(mp, k, y, first, last):
                    nc.tensor.matmul(mp[:], wmix[:, k * C:(k + 1) * C], y[:],
                                     start=first, stop=last)

                # ---- o=0 plane ----
                mp = pspool.tile([C, EB], F32, tag="mmps")
                nc.vector.tensor_mul(ya[:], base[0][:], fk[0][:])
                mixmat(mp, 0, ya, True, False)
                nc.vector.tensor_mul(ya[:], c11[:], fk[1][:])
                mixmat(mp, 1, ya, False, False)
                # C22 = sum_v c21_v * R_v
                nc.vector.tensor_mul(ya[:], c21[0][:], R[0][:])
                nc.vector.tensor_mul(tmp[:], c21[1][:], R[1][:])
                nc.vector.tensor_add(ya[:], ya[:], tmp[:])
                nc.vector.tensor_mul(tmp[:], c21[2][:], R[2][:])
                nc.vector.tensor_add(ya[:], ya[:], tmp[:])
                nc.vector.tensor_mul(ya[:], ya[:], fk[2][:])
                mixmat(mp, 2, ya, False, True)
                nc.scalar.copy(msg[0][:], mp[:])
                # ---- o=1 planes: k3 (base0*f3*R_u), k4 (base1_u*f4),
                #                  k5 (C11*f5*R_v), k6 (C21_v*f6) ----
                f3b = wpool.tile([C, EB], F32, tag="f3b")
                nc.vector.tensor_mul(f3b[:], base[0][:], fk[3][:])
                f5c = wpool.tile([C, EB], F32, tag="f5c")
                nc.vector.tensor_mul(f5c[:], c11[:], fk[5][:])
                for t in range(3):
                    mp = pspool.tile([C, EB], F32, tag="mmps")
                    nc.vector.tensor_mul(ya[:], f3b[:], R[t][:])
                    mixmat(mp, 3, ya, True, False)
                    nc.vector.tensor_mul(ya[:], base[1 + t][:], fk[4][:])
                    mixmat(mp, 4, ya, False, False)
                    nc.vector.tensor_mul(ya[:], f5c[:], R[t][:])
                    mixmat(mp, 5, ya, False, False)
                    nc.vector.tensor_mul(ya[:], c21[t][:], fk[6][:])
                    mixmat(mp, 6, ya, False, True)
                    nc.scalar.copy(msg[1 + t][:], mp[:])
                # ---- o=2 planes (u,v): k7 base0*f7*R_u*R_v, k8 base1_u*f8*R_v,
                #                  k9 base2_uv*f9, k10 C21_u*f10*R_v ----
                f7b = wpool.tile([C, EB], F32, tag="f7b")
                nc.vector.tensor_mul(f7b[:], base[0][:], fk[7][:])
                for u in range(3):
                    nc.vector.tensor_mul(ya[:], f7b[:], R[u][:])          # k7 partial
                    nc.vector.tensor_mul(yb[:], base[1 + u][:], fk[8][:])  # k8 partial
                    nc.vector.tensor_mul(yc[:], c21[u][:], fk[10][:])      # k10 partial
                    for v in range(3):
                        p = 4 + u * 3 + v
                        mp = pspool.tile([C, EB], F32, tag="mmps")
                        y2 = wpool.tile([C, EB], F32, tag="y2")
                        nc.vector.tensor_mul(y2[:], ya[:], R[v][:])
                        mixmat(mp, 7, y2, True, False)
                        nc.vector.tensor_mul(y2[:], yb[:], R[v][:])
                        mixmat(mp, 8, y2, False, False)
                        nc.vector.tensor_mul(y2[:], base[4 + u * 3 + v][:], fk[9][:])
                        mixmat(mp, 9, y2, False, False)
                        nc.vector.tensor_mul(y2[:], yc[:], R[v][:])
                        mixmat(mp, 10, y2, False, True)
                        nc.scalar.copy(msg[p][:], mp[:])

                if phase1:
                    # transpose msg planes to rows, one-hot aggregate per subtile
                    for st in range(4):
                        mrows = wpool.tile([128, FEAT], F32, tag="mrows")
                        for p in range(13):
                            mr = pspool.tile([128, C], F32, tag="mr")
                            nc.tensor.transpose(mr[:], msg[p][:, st * 128:(st + 1) * 128],
                                                ident[:C, :C])
                            nc.scalar.copy(mrows[:, p * C:(p + 1) * C], mr[:])
                        S = wpool.tile([128, 128], F32, tag="S", bufs=2)
                        nc.vector.tensor_scalar(S[:], iota[:], wcol[st], None,
                                                ALU.is_equal)
                        ag = papool.tile([128, FEAT], F32, tag="aggps")
                        nc.tensor.matmul(ag[:, :512], S[:], mrows[:, :512],
                                         start=True, stop=True)
                        nc.tensor.matmul(ag[:, 512:], S[:], mrows[:, 512:],
                                         start=True, stop=True)
                        w = wsched[it * 4 + st]
                        nc.vector.tensor_add(aggrows[:, w, :], aggrows[:, w, :], ag[:])
                else:
                    # per-edge gating nonlinearity + residual, write out
                    out = wpool.tile([C, EB], F32, tag="gout")
                    nc.scalar.activation(out[:], msg[0][:], AF.Sigmoid)
                    nc.vector.tensor_mul(out[:], out[:], msg[0][:])
                    nc.vector.tensor_add(out[:], out[:], ept[0][:])
                    nc.sync.dma_start(edgeout_d[0:C, e0:e0 + EB], out[:])
                    for w in (1, 2):
                        nsp = 3 if w == 1 else 9
                        p0 = PLANE_OFF[w]
                        nrm = wpool.tile([C, EB], F32, tag="nrm")
                        sqp = wpool.tile([C, EB], F32, tag="sqp")
                        nc.scalar.square(nrm[:], msg[p0][:])
                        for s in range(1, nsp):
                            nc.scalar.square(sqp[:], msg[p0 + s][:])
                            nc.vector.tensor_add(nrm[:], nrm[:], sqp[:])
                        gp = pspool.tile([C, EB], F32, tag="mmps")
                        nc.tensor.matmul(gp[:], wg[:, (w - 1) * C:w * C], nrm[:],
                                         start=True, stop=True)
                        gate = wpool.tile([C, EB], F32, tag="gate")
                        gpre = wpool.tile([C, EB], F32, tag="gpre")
                        nc.scalar.activation(gpre[:], gp[:], AF.Identity,
                                             bias=bgc[:, w - 1:w])
                        nc.scalar.activation(gate[:], gpre[:], AF.Sigmoid)
                        nc.vector.tensor_mul(gate[:], gate[:], gpre[:])
                        for s in range(nsp):
                            p = p0 + s
                            nc.vector.tensor_mul(msg[p][:], msg[p][:], gate[:])
                            nc.vector.tensor_add(msg[p][:], msg[p][:], ept[p][:])
                            nc.sync.dma_start(edgeout_d[p * C:(p + 1) * C, e0:e0 + EB],
                                              msg[p][:])

            if phase1:
                # node update epilogue per window
                for w in range(NW):
                    nres = wpool.tile([128, FEAT], F32, tag="nres")
                    nc.sync.dma_start(nres[:], noderes_d[w * 128:(w + 1) * 128, :])
                    nout = wpool.tile([128, FEAT], F32, tag="nout")
                    ar = aggrows[:, w, :]
                    # scalar channel: silu(agg0/16)
                    pre0 = wpool.tile([128, C], F32, tag="pre0")
                    nc.scalar.mul(pre0[:], ar[:, :C], 1.0 / NORM)
                    nc.scalar.activation(nout[:, :C], pre0[:], AF.Sigmoid)
                    nc.vector.tensor_mul(nout[:, :C], nout[:, :C], pre0[:])
                    for ww in (1, 2):
                        nsp = 3 if ww == 1 else 9
                        p0 = PLANE_OFF[ww]
                        nrm = wpool.tile([128, C], F32, tag="wnrm")
                        sqp = wpool.tile([128, C], F32, tag="wsqp")
                        nc.scalar.activation(nrm[:], ar[:, p0 * C:(p0 + 1) * C],
                                             AF.Square, scale=1.0 / NORM)
                        for s in range(1, nsp):
                            nc.scalar.activation(sqp[:], ar[:, (p0 + s) * C:(p0 + s + 1) * C],
                                                 AF.Square, scale=1.0 / NORM)
                            nc.vector.tensor_add(nrm[:], nrm[:], sqp[:])
                        ntp = pspool.tile([C, 128], F32, tag="tp")
                        nc.tensor.transpose(ntp[:], nrm[:], ident[:])
                        nrmT = wpool.tile([C, 128], F32, tag="nrmT")
                        nc.scalar.copy(nrmT[:], ntp[:])
                        gps = pspool.tile([128, C], F32, tag="mr")
                        nc.tensor.matmul(gps[:], nrmT[:], wg[:, (ww - 1) * C:ww * C],
                                         start=True, stop=True)
                        gate = wpool.tile([128, C], F32, tag="wgate")
                        gpre1 = wpool.tile([128, C], F32, tag="gpre1")
                        nc.vector.tensor_add(gpre1[:], gps[:],
                                             bgb[:, (ww - 1) * C:ww * C])
                        nc.scalar.activation(gate[:], gpre1[:], AF.Sigmoid)
                        nc.vector.tensor_mul(gate[:], gate[:], gpre1[:])
                        for s in range(nsp):
                            p = p0 + s
                            nc.vector.scalar_tensor_tensor(
                                nout[:, p * C:(p + 1) * C], ar[:, p * C:(p + 1) * C],
                                1.0 / NORM, gate[:], ALU.mult, ALU.mult)
                    nc.vector.tensor_add(nout[:], nout[:], nres[:])
                    nc.sync.dma_start(nodeout_d[w * 128:(w + 1) * 128, :], nout[:])
    nc.finalize()
    return nc


def _prep_common(eid, idx_j, rij, edgeF, epad):
    """Build per-core streaming inputs from a (padded, -1 marked) edge id list."""
    pad = eid < 0
    eid2 = np.where(pad, 0, eid)
    et = edgeF[eid2]
    et[pad] = 0.0
    rr = rij[eid2].astype(np.float32).copy()
    rr[pad] = np.array([1e6, 0, 0], np.float32)
    gj = np.where(pad, 0, idx_j[eid2]).astype(np.int16)
    return (np.ascontiguousarray(et.T), np.ascontiguousarray(rr),
            _wrap16(gj, epad))


def _record(res):
    global LAST_EXEC_NS
    t = getattr(res, "exec_time_ns", None)
    if t is not None:
        LAST_EXEC_NS = (LAST_EXEC_NS or 0) + t
    p = getattr(res, "profile_json", None)
    if p is not None:
        LAST_PROFILES.append(p)


def kernel(node0, node1, node2, edge0, edge1, edge2, idx_i, idx_j, rij,
           Wrbf_n, Wmix_n, Wg_n, bg_n, Wrbf_e, Wmix_e, Wg_e, bg_e):
    idx_i = np.asarray(idx_i); idx_j = np.asarray(idx_j)
    rij = np.asarray(rij, np.float32)
    E = idx_i.shape[0]
    nodesF = _pack_feat(np.asarray(node0), np.asarray(node1), np.asarray(node2))
    edgeF = _pack_feat(np.asarray(edge0), np.asarray(edge1), np.asarray(edge2))

    ident = np.eye(128, dtype=np.float32)
    iota = np.tile(np.arange(128, dtype=np.float32), (128, 1))
    centers = np.tile(np.linspace(0, RMAX, NB, dtype=np.float32), (128, 1))

    # ---------- phase 1: bucket edges by (core, window) of idx_i ----------
    part = idx_i // NPC
    loc = idx_i % NPC
    win = loc // 128
    wloc = loc % 128
    cnt = np.zeros((NCORES, NW), np.int64)
    np.add.at(cnt, (part, win), 1)
    budget = ((cnt.max(axis=0) + 127) // 128) * 128
    tot = int(budget.sum())
    epad1 = ((tot + EB - 1) // EB) * EB
    wsched = []
    for w in range(NW):
        wsched += [w] * (int(budget[w]) // 128)
    wsched += [0] * ((epad1 - tot) // 128)

    order = np.argsort(part * NW + win, kind="stable")
    eids = [np.full(epad1, -1, np.int64) for _ in range(NCORES)]
    off = np.zeros(NW + 1, np.int64)
    off[1:] = np.cumsum(budget)
    pos = 0
    for c in range(NCORES):
        for w in range(NW):
            k = int(cnt[c, w])
            eids[c][off[w]:off[w] + k] = order[pos:pos + k]
            pos += k

    nc1 = build_conv(epad1, wsched, True)
    LAST_NCS.clear()
    LAST_NCS.append(nc1)
    in_maps = []
    for c in range(NCORES):
        et, rr, gj = _prep_common(eids[c], idx_j, rij, edgeF, epad1)
        wl = np.where(eids[c] < 0, 0, wloc[np.where(eids[c] < 0, 0, eids[c])])
        wlcol = np.ascontiguousarray(wl.astype(np.float32).reshape(epad1 // 128, 128).T)
        nres = np.zeros((NPAD, FEAT), np.float32)
        nres[:NPC] = nodesF[c * NPC:(c + 1) * NPC]
        in_maps.append(dict(
            nodes_all=nodesF, edgeT=et, rijrows=rr, gidx=gj,
            widx=wlcol,
            noderes=nres,
            wrbf=np.asarray(Wrbf_n, np.float32), wmix=np.asarray(Wmix_n, np.float32),
            wg=np.asarray(Wg_n, np.float32),
            bgb=np.tile(np.asarray(bg_n, np.float32)[:, None, :], (1, 128, 1)),
            ident=ident, centers=centers, iota=iota,
        ))
    res1 = bass_utils.run_bass_kernel_spmd(nc1, in_maps, list(range(NCORES)))
    _record(res1)
    nodesNewF = np.concatenate(
        [res1.results[c]["nodeout"][:NPC] for c in range(NCORES)], axis=0)

    # ---------- phase 2: positional edge shard, edge update ----------
    epc = E // NCORES
    epad2 = ((epc + EB - 1) // EB) * EB
    nc2 = build_conv(epad2, None, False)
    LAST_NCS.append(nc2)
    in_maps = []
    for c in range(NCORES):
        eid = np.full(epad2, -1, np.int64)
        eid[:epc] = np.arange(c * epc, (c + 1) * epc)
        et, rr, gj = _prep_common(eid, idx_j, rij, edgeF, epad2)
        in_maps.append(dict(
            nodes_all=nodesNewF, edgeT=et, rijrows=rr, gidx=gj,
            wrbf=np.asarray(Wrbf_e, np.float32), wmix=np.asarray(Wmix_e, np.float32),
            wg=np.asarray(Wg_e, np.float32),
            bgc=np.ascontiguousarray(np.asarray(bg_e, np.float32).T),
            ident=ident, centers=centers,
        ))
    res2 = bass_utils.run_bass_kernel_spmd(nc2, in_maps, list(range(NCORES)))
    _record(res2)
    edgeNewF = np.concatenate(
        [res2.results[c]["edgeoutT"].T[:epc] for c in range(NCORES)], axis=0)

    n0, n1, n2 = _unpack_feat(nodesNewF)
    e0, e1, e2 = _unpack_feat(edgeNewF)
    return (n0, n1, n2, e0, e1, e2)
